# revision 29
# baseline (speedup 1.0000x reference)
"""Closed-form kinetic-optimal Euler row kernel.

Algebra: with i=x_t[n], m=x_1[n], eq=(i==m), the reference row collapses to

    row = eq*(w - delta_i * S_w) + (1-eq) * v * (delta_m - delta_i)

where w_j  = relu(-dk*s_j) / ((1-k)*s_j + eps)           (token-independent),
      S_w  = sum_j w_j,
      v    = relu(dk*s_i) / ((1-k)*s_m + k + eps)        (per-token scalar).

Proof sketch: for j not in {i,m} the antisymmetric numerator reduces to
e_j = -s_j*dk*eq, so off-one-hot entries vanish whenever i != m; the j=m
entry simplifies to dk*s_i, and the diagonal entry is minus the row sum.

Device program (per core, 64 tokens, 2 partitions/token, 256-wide halves):
    io  = prefix-scan iota            memset + tensor_tensor_scan (no DMA dep)
    bdc = (io == x1') * beta          one tensor_scalar (is_equal, mult)
    gdi = (io == xt') * (-gamma)      one tensor_scalar (is_equal, mult)
    row = bdc + gdi                   one tensor_tensor
with beta = (1-eq)*v and gamma = eq*S_w + (1-eq)*v as per-partition fp32
scalar columns (fp16 everywhere else). The input DMA carries only 32B/row
(4 fp32 scalars + 8 int16 scatter indices, bitcast into fp16 slots). The
output leaves through a PREPARE_ONLY dma_scatter_add (identity row indices,
pre-zeroed DRAM dest): its descriptor generation overlaps the DVE compute,
and only the trigger + transfer + DMA-sem sit on the tail. A second module
variant adds the w-tile term (only needed when dk<0 AND some token has
x_t==x_1) through the plain HWDGE output path in fp32.
"""
import numpy as np
from contextlib import ExitStack

N = 512
V = 512
NCORES = 8
NT = N // NCORES  # 64 tokens/core
P = 2 * NT        # 128 partitions, 2 per token
H = V // 2        # 256 free elements (half vocab per partition)
EPS = 1e-8
NSC = 4           # scalar columns: xt', x1', beta, -gamma
IDXS = 8          # int16 scatter indices (8 per partition, replicated blocks)


def _rewire_swdge_completion(nc, prep_ins):
    """Make the DMASW lane sem reflect TRUE scatter-DMA completion for the
    prepare/trigger path.

    Tile's bookkeeping for a gen_mode==1 SWDGE prep bumps its DMASW lane sem
    with a Pool-side InstIncSwdgeSem at PREP time (when only the descriptors
    are written), while the DMA descriptor's completion bump goes to the
    private sem= semaphore nobody waits on. End-of-kernel cleanup waits on
    the DMASW lane sem, so the kernel could retire while the triggered
    scatter is still in flight, and the cost-model sim deadlocks because
    InstIncSwdgeSem's side bump is not modeled. Fix both by (1) encoding the
    DMASW lane sem into the descriptor (on_update[0]) so SDMA bumps it at
    completion, and (2) removing the prep-time InstIncSwdgeSem bump."""
    dmasw = None
    removed = 0
    for blk in nc.m.functions[0].blocks:
        il = blk.instructions
        keep = [i for i in il if type(i).__name__ != "InstIncSwdgeSem"]
        if len(keep) != len(il):
            removed += len(il) - len(keep)
            blk.instructions = keep
        for ins in keep:
            si = ins.sync_info
            if si is None:
                continue
            for x in si.on_wait:
                if x.ant_name and x.ant_name.startswith("DMASW"):
                    dmasw = x
    assert removed == 1, f"expected exactly one IncSwdgeSem pre-bump, removed {removed}"
    assert dmasw is not None, "no DMASW waiter found to rewire"
    u0 = prep_ins.sync_info.on_update[0]
    assert u0.ant_name == "wb_dma", u0
    u0.id = dmasw.id
    try:
        u0.ant_name = dmasw.ant_name
    except Exception:
        pass


def build(wvar: bool, mode: str = "scatter"):
    """mode: 'scatter' = prepared dma_scatter_add output (fast path),
    'hwdge' = plain dma_start output."""
    import concourse.mybir as mybir
    from concourse import bacc
    from concourse import tile

    Alu = mybir.AluOpType
    fp32 = mybir.dt.float32
    fp16 = mybir.dt.float16
    i16 = mybir.dt.int16
    dt = fp32 if wvar else fp16
    if wvar:
        mode = "hwdge"
    dev_idx = mode == "scatter"  # build scatter indices on DVE, off the DMA
    # scalar columns are always fp32 (is_equal requires fp32 scalars); in the
    # fp16 variant they occupy 2*NSC fp16 slots via bitcast
    sc_slots = NSC if wvar else 2 * NSC
    idx_slots = IDXS if (mode == "scatter" and not dev_idx) else 0

    nc = bacc.Bacc("TRN2", target_bir_lowering=False, debug=False)

    Wtot = sc_slots + idx_slots + (H if wvar else 0)
    pk_d = nc.dram_tensor("pk", [P, Wtot], dt, kind="ExternalInput")
    out_d = nc.dram_tensor("out", [NT, V], dt, kind="ExternalOutput")

    with tile.TileContext(nc) as tc, ExitStack() as ctx:
        pool = ctx.enter_context(tc.tile_pool(name="main", bufs=1))

        # iota rows built on DVE while the input DMA is in flight:
        # state = (1 + state) with initial -1 gives 0..H-1 exactly (fp32 state)
        ones = pool.tile([P, H], dt, name="ones")
        io_f = pool.tile([P, H], dt, name="io_f")
        nc.vector.memset(ones[:], 1.0)
        nc.vector.tensor_tensor_scan(
            io_f[:], ones[:], ones[:], -1.0, Alu.add, Alu.bypass
        )
        io_t = io_f[:]

        if dev_idx:
            # scatter idx table val(p,f) = p%16 + 16f, built with no DMA dep
            # so the SWDGE prep's descriptor generation overlaps the input
            # DMA latency. Stream-transpose of the iota gives out[p,:]=p%32.
            i16t = mybir.dt.int16
            tr = pool.tile([P, 32], dt, name="tr")
            nc.vector.transpose(tr[:], io_f[:, 0:32])
            trc = pool.tile([P, 1], fp32, name="trc")
            g16 = pool.tile([P, 1], fp32, name="g16")
            pm16 = pool.tile([P, 1], fp32, name="pm16")
            idx_f = pool.tile([P, IDXS], fp32, name="idx_f")
            idx_t = pool.tile([P, IDXS], i16t, name="idx_t")
            nc.vector.tensor_scalar(trc[:], tr[:, 0:1], 0.0, None, Alu.add)
            nc.vector.tensor_scalar(g16[:], trc[:], 16.0, 16.0, Alu.is_ge, Alu.mult)
            nc.vector.tensor_tensor(pm16[:], trc[:], g16[:], Alu.subtract)
            nc.vector.tensor_scalar(
                idx_f[:], io_f[:, 0:IDXS], 16.0, pm16[:], Alu.mult, Alu.add
            )
            nc.vector.tensor_scalar(idx_t[:], idx_f[:], 0.0, None, Alu.add)
            idxs_ap = idx_t[:]

        pk_t = pool.tile([P, Wtot], dt, name="pk_t")
        nc.sync.dma_start(pk_t[:], pk_d.ap())

        sc = pk_t[:, 0:sc_slots] if wvar else pk_t[:, 0:sc_slots].bitcast(fp32)
        xt_c, x1_c = sc[:, 0:1], sc[:, 1:2]
        b_c, ng_c = sc[:, 2:3], sc[:, 3:4]
        if idx_slots:
            # [128, 8] int16 identity indices; each 16-partition block holds
            # the same wrapped [16, 8] pattern (one per GPSIMD Q7 core)
            idxs_ap = pk_t[:, sc_slots : sc_slots + idx_slots].bitcast(i16)

        bdc = pool.tile([P, H], dt, name="bdc")
        gdi = pool.tile([P, H], dt, name="gdi")
        row = pool.tile([P, H], dt, name="row")

        nc.vector.tensor_scalar(bdc[:], io_t, x1_c, b_c, Alu.is_equal, Alu.mult)
        nc.vector.tensor_scalar(gdi[:], io_t, xt_c, ng_c, Alu.is_equal, Alu.mult)
        if wvar:
            w_t = pk_t[:, sc_slots + idx_slots :]
            r1 = pool.tile([P, H], dt, name="r1")
            nc.vector.tensor_tensor(r1[:], w_t, bdc[:], Alu.add)
            nc.vector.tensor_tensor(row[:], r1[:], gdi[:], Alu.add)
        else:
            nc.vector.tensor_tensor(row[:], bdc[:], gdi[:], Alu.add)

        out_ap = out_d.ap().rearrange("a (h b) -> (a h) b", h=2)
        if mode == "scatter":
            # out[idxs, :] += row with identity indices and a pre-zeroed
            # DRAM dest == plain write; src RAW defers to the trigger so the
            # prep's descriptor generation overlaps the input DMA + compute
            wb_sem = nc.alloc_semaphore("wb_dma")
            prep = nc.gpsimd.dma_scatter_add(
                out_ap,
                row[:].unsqueeze(1),
                idxs_ap,
                P,
                P,
                H,
                prepare_only=True,
                sem=wb_sem,
            )
            nc.gpsimd.trigger_dma(count=None)
        else:
            nc.sync.dma_start(out_ap, row[:])

    if mode == "scatter":
        _rewire_swdge_completion(nc, prep.ins if hasattr(prep, "ins") else prep)

    nc.compile()
    return nc


def _host_scalars(source_p, k_t, d_k_t, x_t, x_1):
    s = np.asarray(source_p, dtype=np.float64).reshape(V)
    k = float(np.asarray(k_t).reshape(()))
    dk = float(np.asarray(d_k_t).reshape(()))
    xt = np.asarray(x_t).reshape(N).astype(np.int64)
    x1 = np.asarray(x_1).reshape(N).astype(np.int64)

    eq = xt == x1
    v = np.maximum(dk * s[xt], 0.0) / ((1.0 - k) * s[x1] + k + EPS)
    w = np.maximum(-dk * s, 0.0) / ((1.0 - k) * s + EPS)
    S_w = float(w.sum())
    beta = np.where(eq, 0.0, v)
    gamma = np.where(eq, S_w, v)
    wvar = bool(w.any() and eq.any())
    return s, xt, x1, eq, beta, gamma, w, wvar


def in_maps(source_p, k_t, d_k_t, x_t, x_1, mode: str = "scatter"):
    s, xt, x1, eq, beta, gamma, w, wvar = _host_scalars(
        source_p, k_t, d_k_t, x_t, x_1
    )
    npdt = np.float32 if wvar else np.float16
    if wvar:
        mode = "hwdge"
    sc_slots = NSC if wvar else 2 * NSC
    idx_slots = 0  # scatter indices are built on-device
    Wtot = sc_slots + idx_slots + (H if wvar else 0)
    parity = np.tile(np.array([0, 1], dtype=np.int64), NT)  # per partition

    base = np.zeros((P, Wtot), dtype=npdt)
    if idx_slots:
        idx = np.arange(P, dtype=np.int16).reshape(IDXS, 16).T  # [16, 8] wrapped
        base[:, sc_slots : sc_slots + idx_slots] = np.tile(idx, (P // 16, 1)).view(
            np.float16
        )

    maps = []
    for c in range(NCORES):
        lo, hi = c * NT, (c + 1) * NT
        pk = base.copy()
        sc = np.empty((P, NSC), dtype=np.float32)
        sc[:, 0] = np.repeat(xt[lo:hi], 2) - H * parity
        sc[:, 1] = np.repeat(x1[lo:hi], 2) - H * parity
        sc[:, 2] = np.repeat(beta[lo:hi], 2)
        sc[:, 3] = np.repeat(-gamma[lo:hi], 2)
        if wvar:
            pk[:, 0:NSC] = sc
            wtile = np.where(
                np.repeat(eq[lo:hi], 2)[:, None],
                np.stack([w[:H], w[H:]], axis=0)[parity],
                0.0,
            )
            pk[:, sc_slots + idx_slots :] = wtile
        else:
            pk[:, 0 : 2 * NSC] = sc.view(np.float16)
        maps.append({"pk": pk})
    return maps, wvar


_CACHE = {}
_MODE = {"mode": "scatter"}


def _get_nc(wvar: bool = False):
    key = ("nc", wvar, _MODE["mode"])
    if key not in _CACHE:
        try:
            _CACHE[key] = build(wvar, mode=_MODE["mode"])
        except Exception:
            if _MODE["mode"] == "scatter":
                # prepared-scatter path failed to build (framework drift?):
                # fall back to the plain HWDGE output, which is slower but
                # uses only vanilla Tile codegen
                _MODE["mode"] = "hwdge"
                key = ("nc", wvar, "hwdge")
                if key not in _CACHE:
                    _CACHE[key] = build(wvar, mode="hwdge")
            else:
                raise
    return _CACHE[key]


def _in_maps(source_p, k_t, d_k_t, x_t, x_1):
    return in_maps(source_p, k_t, d_k_t, x_t, x_1, mode=_MODE["mode"])[0]


def kernel(source_p, k_t, d_k_t, x_t, x_1):
    from concourse.bass_utils import run_bass_kernel_spmd

    maps, wvar = in_maps(source_p, k_t, d_k_t, x_t, x_1, mode=_MODE["mode"])
    nc = _get_nc(wvar)
    res = run_bass_kernel_spmd(nc, maps, list(range(NCORES)))
    out = np.concatenate([res.results[c]["out"] for c in range(NCORES)], axis=0)
    return out.astype(np.float32)
